# revision 4
# baseline (speedup 1.0000x reference)
"""LinearAttention Trainium2 kernel (8 NeuronCores, batch+sequence sharded).

Reference computation (per batch b):
    qkv = x @ W_qkv; q,k,v split; per-head: softmax(q, dim=dh),
    softmax(k, dim=seq); ctx = k^T v; out = q_sm @ ctx; y = out @ W_out + b.

Sharding: cores 0-3 hold batch 0, cores 4-7 batch 1; each core owns 2048
sequence rows.  The k-softmax/ctx reduction over the full sequence is an
AllReduce within each 4-core group (the two groups run concurrently).

Device dataflow per core (16 tiles of 128 seq rows):
  phase 1: qkv = xt.T @ Wq (bf16 matmuls, f32 PSUM); exp_q via one ACT
           instruction; per-head sums via one grouped DVE reduce;
           q_sm = exp_q * (1/sum) via one broadcast-AP DVE multiply;
           q_sm transposed into qsmT via DMA-XBAR (not the PE)
  phase 2: head-pair gram blocks [v_h0|v_h1]^T [ek_h0|ek_h1] (diagonal
           64x64 sub-blocks are the per-head ctx^T) + Z col-sums,
           accumulated in SBUF f32
  phase 3: one 132KB bf16 AllReduce of [ctx-pairs | Z] per 4-core group
  phase 4: M_h = (1/Z_h) * ctx_h @ W_out_h, with the 1/Z row-scaling
           folded into the PSUM->SBUF copy on the ACT engine
  phase 5: y = sum_t qsmT_t.T @ M_t (full-rate 128-contraction matmuls),
           y emitted bf16
Host: shards/transposes/casts x, gathers per-core y shards, adds b_out.
"""
import numpy as np
import ml_dtypes
from contextlib import ExitStack

import concourse.bass as bass
import concourse.mybir as mybir
import concourse.tile as tile
from concourse import bacc
from concourse.bass_utils import run_bass_kernel_spmd

bf16 = ml_dtypes.bfloat16
F32 = mybir.dt.float32
BF = mybir.dt.bfloat16
EXP = mybir.ActivationFunctionType.Exp
COPY = mybir.ActivationFunctionType.Copy

B, N, D = 2, 8192, 1024
H, DH, INNER = 8, 64, 512
NCORES = 8
GROUP = 4                   # cores per batch
SEQ = N // GROUP            # 2048 seq rows per core (one batch)
NT = SEQ // 128             # 16 seq tiles


def _body(tc, xT, wq, wo, y):
    nc = tc.nc
    with ExitStack() as ctx:
        const = ctx.enter_context(tc.tile_pool(name="const", bufs=1))
        dram = ctx.enter_context(tc.tile_pool(name="dram", bufs=1, space="DRAM"))

        ones_bf = const.tile([128, 1], BF)
        nc.vector.memset(ones_bf, 1.0)

        # per-k-block weight tiles for fine-grained startup deps
        wq_sb = []
        for kk in range(8):
            t = const.tile([128, 3 * INNER], BF, tag=f"wq{kk}")
            nc.sync.dma_start(out=t, in_=wq[128 * kk:128 * (kk + 1), :])
            wq_sb.append(t)
        wo_sb = const.tile([128, 4, D], BF)
        for t in range(4):
            nc.sync.dma_start(out=wo_sb[:, t, :], in_=wo[128 * t:128 * (t + 1), :])

        qsmT = const.tile([128, 4, SEQ], BF)  # persistent q_sm^T
        xT_r = xT[:].rearrange("(c p) s -> p c s", p=128)  # [128, 8, 2048]

        cz_acc = const.tile([128, 516], F32)  # [ctx head-pairs | Z]
        nc.vector.memset(cz_acc, 0.0)

        with ExitStack() as p12:
            xt_pool = p12.enter_context(tc.tile_pool(name="xt", bufs=3))
            work = p12.enter_context(tc.tile_pool(name="work", bufs=3))
            small = p12.enter_context(tc.tile_pool(name="small", bufs=4))
            qk_psum = p12.enter_context(tc.tile_pool(name="qk_ps", bufs=2, space="PSUM"))
            v_psum = p12.enter_context(tc.tile_pool(name="v_ps", bufs=1, space="PSUM"))
            c_psum = p12.enter_context(tc.tile_pool(name="c_ps", bufs=1, space="PSUM"))
            z_psum = p12.enter_context(tc.tile_pool(name="z_ps", bufs=1, space="PSUM"))

            for m in range(NT):
                xt = xt_pool.tile([128, 8, 128], BF, tag="xt")
                nc.sync.dma_start(out=xt, in_=xT_r[:, :, m * 128:(m + 1) * 128])

                qkv_ps = qk_psum.tile([128, 1024], F32, tag="qk")
                v_ps = v_psum.tile([128, 512], F32, tag="vp")
                for kk in range(8):
                    nc.tensor.matmul(
                        qkv_ps[:, 0:512], lhsT=xt[:, kk, :],
                        rhs=wq_sb[kk][:, 0:512],
                        start=(kk == 0), stop=(kk == 7))
                    nc.tensor.matmul(
                        qkv_ps[:, 512:1024], lhsT=xt[:, kk, :],
                        rhs=wq_sb[kk][:, 512:1024],
                        start=(kk == 0), stop=(kk == 7))
                    nc.tensor.matmul(
                        v_ps, lhsT=xt[:, kk, :],
                        rhs=wq_sb[kk][:, 1024:1536],
                        start=(kk == 0), stop=(kk == 7))

                # q softmax over dh: one exp, one grouped reduce, one
                # broadcast multiply
                expq = work.tile([128, INNER], F32, tag="expq")
                nc.scalar.activation(out=expq, in_=qkv_ps[:, 0:512], func=EXP)
                qs = small.tile([128, 8], F32, tag="qs")
                nc.vector.tensor_reduce(
                    out=qs, in_=expq[:].rearrange("p (h d) -> p h d", h=H),
                    axis=mybir.AxisListType.X, op=mybir.AluOpType.add)
                rq = small.tile([128, 8], F32, tag="rq")
                nc.vector.reciprocal(rq, qs)
                qsm = work.tile([128, INNER], BF, tag="qsm")
                nc.vector.tensor_tensor(
                    out=qsm[:].rearrange("p (h d) -> p h d", h=H),
                    in0=expq[:].rearrange("p (h d) -> p h d", h=H),
                    in1=rq[:].broadcast_to([128, H, DH]),
                    op=mybir.AluOpType.mult)

                expk = work.tile([128, INNER], BF, tag="expk")
                nc.scalar.activation(out=expk, in_=qkv_ps[:, 512:1024], func=EXP)
                vsb = work.tile([128, INNER], BF, tag="v")
                nc.scalar.activation(out=vsb, in_=v_ps, func=COPY)

                # q_sm^T via DMA XBAR transpose (128x128 bf16 blocks)
                for t4 in range(4):
                    nc.sync.dma_start(
                        out=qsmT[:, t4, m * 128:(m + 1) * 128],
                        in_=qsm[:, 128 * t4:128 * (t4 + 1)], transpose=True)

                # head-pair gram blocks + Z col-sums
                cz = c_psum.tile([128, 512], F32, tag="cz")
                for p in range(4):
                    nc.tensor.matmul(
                        cz[:, 128 * p:128 * (p + 1)],
                        lhsT=vsb[:, 128 * p:128 * (p + 1)],
                        rhs=expk[:, 128 * p:128 * (p + 1)],
                        start=True, stop=True)
                zp = z_psum.tile([128, 4], F32, tag="zp")
                for j in range(4):
                    nc.tensor.matmul(
                        zp[:, j:j + 1],
                        lhsT=expk[:, 128 * j:128 * (j + 1)], rhs=ones_bf,
                        start=True, stop=True)
                nc.vector.tensor_add(cz_acc[:, 0:512], cz_acc[:, 0:512], cz)
                nc.vector.tensor_add(cz_acc[:, 512:516], cz_acc[:, 512:516], zp)

        # bf16 collective payload
        cz_bf = const.tile([128, 516], BF)
        nc.vector.tensor_copy(out=cz_bf, in_=cz_acc)
        part = dram.tile([128, 516], BF, tag="part")
        red = dram.tile([128, 516], BF, tag="red")
        nc.sync.dma_start(out=part, in_=cz_bf)
        nc.gpsimd.collective_compute(
            "AllReduce", mybir.AluOpType.add,
            replica_groups=[[0, 1, 2, 3], [4, 5, 6, 7]],
            ins=[part.opt()], outs=[red.opt()])

        with ExitStack() as p45:
            work2 = p45.enter_context(tc.tile_pool(name="work2", bufs=2))
            small2 = p45.enter_context(tc.tile_pool(name="small2", bufs=2))
            ysb_pool = p45.enter_context(tc.tile_pool(name="ysb", bufs=4))
            m_psum = p45.enter_context(tc.tile_pool(name="m_ps", bufs=2, space="PSUM"))
            y_psum = p45.enter_context(tc.tile_pool(name="y_ps", bufs=4, space="PSUM"))

            red_sb = work2.tile([128, 516], BF, tag="red")
            nc.sync.dma_start(out=red_sb, in_=red)
            rz = small2.tile([128, 4], F32, tag="rz")
            nc.vector.reciprocal(rz, red_sb[:, 512:516])

            # M_h = (1/Z) * ctx_h @ Wout_h; 1/Z folded into the ACT copy
            m_sb = work2.tile([128, 4, D], BF, tag="msb")
            for p in range(4):
                for cb in range(2):
                    mp = m_psum.tile([128, 512], F32, tag="mp")
                    for j in range(2):
                        nc.tensor.matmul(
                            mp[64 * j:64 * (j + 1), :],
                            lhsT=red_sb[64 * j:64 * (j + 1),
                                        128 * p + 64 * j:128 * p + 64 * (j + 1)],
                            rhs=wo_sb[64 * j:64 * (j + 1), p,
                                      cb * 512:(cb + 1) * 512],
                            start=True, stop=True)
                    nc.scalar.activation(
                        out=m_sb[:, p, cb * 512:(cb + 1) * 512], in_=mp,
                        func=COPY, scale=rz[:, p:p + 1])

            # y = sum_t qsmT_t^T @ M_t
            for mi in range(NT):
                for cb in range(2):
                    yp = y_psum.tile([128, 512], F32, tag="yp")
                    for t in range(4):
                        nc.tensor.matmul(
                            yp, lhsT=qsmT[:, t, mi * 128:(mi + 1) * 128],
                            rhs=m_sb[:, t, cb * 512:(cb + 1) * 512],
                            start=(t == 0), stop=(t == 3))
                    ysb = ysb_pool.tile([128, 512], BF, tag="ysb")
                    if cb == 0:
                        nc.vector.tensor_copy(out=ysb, in_=yp)
                    else:
                        nc.scalar.activation(out=ysb, in_=yp, func=COPY)
                    nc.sync.dma_start(
                        out=y[mi * 128:(mi + 1) * 128,
                              cb * 512:(cb + 1) * 512],
                        in_=ysb)


def _install_ntff_shim():
    """Provide antenv.axon_hooks when the image lacks it, so
    run_bass_kernel_spmd(trace=True) can capture NTFF profiles."""
    import sys as _sys
    try:
        import antenv.axon_hooks  # noqa: F401
        return
    except ImportError:
        pass
    try:
        import types
        from trn_agent_boot.trn_boot import _ntff_profile_via_ctypes
        hook = _ntff_profile_via_ctypes("/opt/axon/libaxon_pjrt.so")
        mod = types.ModuleType("antenv.axon_hooks")
        mod._hook = hook
        mod.get_axon_ntff_profile_hook = lambda: mod._hook
        def _set(h):
            mod._hook = h
        mod.set_axon_ntff_profile_hook = _set
        _sys.modules["antenv.axon_hooks"] = mod
    except Exception:
        pass


_COMPILED = None


def _build():
    global _COMPILED
    if _COMPILED is None:
        nc = bacc.Bacc("TRN2", target_bir_lowering=False, debug=False,
                       num_devices=NCORES)
        xT = nc.declare_dram_parameter("xT", [D, SEQ], BF, isOutput=False)
        wq = nc.declare_dram_parameter("wq", [D, 3 * INNER], BF, isOutput=False)
        wo = nc.declare_dram_parameter("wo", [INNER, D], BF, isOutput=False)
        y = nc.declare_dram_parameter("y", [SEQ, D], BF, isOutput=True)
        with tile.TileContext(nc) as tc:
            _body(tc, xT, wq, wo, y)
        nc.compile()
        _COMPILED = nc
    return _COMPILED


def _make_in_maps(x, W_qkv, W_out):
    wq_bf = np.ascontiguousarray(W_qkv).astype(bf16)
    wo_bf = np.ascontiguousarray(W_out).astype(bf16)
    in_maps = []
    for c in range(NCORES):
        b = c // GROUP
        r0 = (c % GROUP) * SEQ
        xT_bf = np.ascontiguousarray(x[b, r0:r0 + SEQ].T).astype(bf16)
        in_maps.append({"xT": xT_bf, "wq": wq_bf, "wo": wo_bf})
    return in_maps


def _run(x, W_qkv, W_out, b_out, trace=False, **spmd_kwargs):
    nc = _build()
    in_maps = _make_in_maps(x, W_qkv, W_out)
    res = run_bass_kernel_spmd(nc, in_maps, list(range(NCORES)),
                               trace=trace, **spmd_kwargs)
    out = np.empty((B, N, D), np.float32)
    for c in range(NCORES):
        b = c // GROUP
        r0 = (c % GROUP) * SEQ
        out[b, r0:r0 + SEQ] = res.results[c]["y"].astype(np.float32)
    out += np.asarray(b_out, np.float32)[None, None, :]
    return out, res


def kernel(x, W_qkv, W_out, b_out):
    x = np.asarray(x, np.float32)
    out, _ = _run(x, np.asarray(W_qkv, np.float32),
                  np.asarray(W_out, np.float32),
                  np.asarray(b_out, np.float32))
    return out


# revision 21
# speedup vs baseline: 1.3241x; 1.3241x over previous
"""LinearAttention Trainium2 kernel (8 NeuronCores, batch+sequence sharded).

Reference computation (per batch b):
    qkv = x @ W_qkv; q,k,v split; per-head: softmax(q, dim=dh),
    softmax(k, dim=seq); ctx = k^T v; out = q_sm @ ctx; y = out @ W_out + b.

Sharding: cores 0-3 hold batch 0, cores 4-7 batch 1; each core owns 2048
sequence rows.  The k-softmax/ctx reduction over the full sequence is an
AllReduce within each 4-core group (the two groups run concurrently).

Device dataflow per core (16 tiles of 128 seq rows):
  phase 1: qkv = xt.T @ Wq (bf16 matmuls, f32 PSUM); exp_q via one ACT
           instruction; per-head sums via one grouped DVE reduce;
           q_sm = exp_q * (1/sum) via one broadcast-AP DVE multiply;
           q_sm transposed into qsmT via PE transposes
  DMA queues: weights on scalar, x tiles on sync, y out on vector,
           reduction payloads on gpsimd — four parallel queues
  phase 2: head-pair gram blocks [v_h0|v_h1]^T [ek_h0|ek_h1] (diagonal
           64x64 sub-blocks are the per-head ctx^T) + Z col-sums,
           accumulated in SBUF f32
  phase 3: one 132KB bf16 AllReduce of [ctx-pairs | Z] per 4-core group
  phase 4: M_h = (1/Z_h) * ctx_h @ W_out_h, with the 1/Z row-scaling
           folded into the PSUM->SBUF copy on the ACT engine
  phase 5: y = sum_t qsmT_t.T @ M_t (full-rate 128-contraction matmuls),
           y emitted bf16
Host: shards/transposes/casts x, gathers per-core y shards, adds b_out.
"""
import numpy as np
import ml_dtypes
from contextlib import ExitStack

import concourse.bass as bass
import concourse.mybir as mybir
import concourse.tile as tile
from concourse import bacc
from concourse.bass_utils import run_bass_kernel_spmd

_FAKE = []  # (BassInstruction, sem_name) neutered after scheduling


def _fake_inc(engine, sem, val):
    """Attach a scheduler-only local increment to an externally-incremented
    semaphore so the tile scheduler's single-core simulation doesn't
    deadlock; stripped from the instruction before compile."""
    inst = engine.wait_ge(sem, 0)
    inst.then_inc(sem, val)
    _FAKE.append((inst, sem.name))
    return inst


def _neuter_fakes():
    for inst, sem_name in _FAKE:
        si = inst.ins.sync_info
        si.on_update = [u for u in si.on_update if u.ant_name != sem_name]
    _FAKE.clear()

bf16 = ml_dtypes.bfloat16
F32 = mybir.dt.float32
BF = mybir.dt.bfloat16
EXP = mybir.ActivationFunctionType.Exp
COPY = mybir.ActivationFunctionType.Copy

B, N, D = 2, 8192, 1024
H, DH, INNER = 8, 64, 512
NCORES = 8
GROUP = 4                   # cores per batch
SEQ = N // GROUP            # 2048 seq rows per core (one batch)
NT = SEQ // 128             # 16 seq tiles


def _body(tc, xT, wq, wo, y):
    nc = tc.nc
    rsem = nc.alloc_semaphore("cz_rsem")
    lsem = nc.alloc_semaphore("cz_lsem")
    with ExitStack() as ctx:
        const = ctx.enter_context(tc.tile_pool(name="const", bufs=1))
        dram = ctx.enter_context(tc.tile_pool(name="dram", bufs=1, space="DRAM"))

        ones_bf = const.tile([128, 1], BF)
        nc.vector.memset(ones_bf, 1.0)
        ident = const.tile([128, 128], BF)
        from concourse.masks import make_identity
        make_identity(nc, ident)

        # per-k-block weight tiles for fine-grained startup deps; weights on
        # the scalar DMA queue so x-tile loads (sync queue) run in parallel
        wq_sb = []
        for kk in range(8):
            t = const.tile([128, 3 * INNER], BF, tag=f"wq{kk}")
            nc.scalar.dma_start(out=t, in_=wq[128 * kk:128 * (kk + 1), :])
            wq_sb.append(t)
        wo_sb = const.tile([128, 4, D], BF)
        for t in range(4):
            nc.scalar.dma_start(out=wo_sb[:, t, :], in_=wo[128 * t:128 * (t + 1), :])

        qsmT = const.tile([128, 4, SEQ], BF)  # persistent q_sm^T
        xT_r = xT[:].rearrange("(c p) s -> p c s", p=128)  # [128, 8, 2048]

        cz_acc = const.tile([128, 516], F32)  # [ctx head-pairs | Z]
        nc.vector.memset(cz_acc, 0.0)

        with ExitStack() as p12:
            xt_pool = p12.enter_context(tc.tile_pool(name="xt", bufs=3))
            work = p12.enter_context(tc.tile_pool(name="work", bufs=3))
            small = p12.enter_context(tc.tile_pool(name="small", bufs=4))
            qk_psum = p12.enter_context(tc.tile_pool(name="qk_ps", bufs=2, space="PSUM"))
            v_psum = p12.enter_context(tc.tile_pool(name="v_ps", bufs=1, space="PSUM"))
            c_psum = p12.enter_context(tc.tile_pool(name="c_ps", bufs=1, space="PSUM"))
            z_psum = p12.enter_context(tc.tile_pool(name="z_ps", bufs=1, space="PSUM"))
            tr_psum = p12.enter_context(tc.tile_pool(name="tr_ps", bufs=1, space="PSUM"))

            for m in range(NT):
                xt = xt_pool.tile([128, 8, 128], BF, tag="xt")
                nc.sync.dma_start(out=xt, in_=xT_r[:, :, m * 128:(m + 1) * 128])

                qkv_ps = qk_psum.tile([128, 1024], F32, tag="qk")
                v_ps = v_psum.tile([128, 512], F32, tag="vp")
                for kk in range(8):
                    nc.tensor.matmul(
                        qkv_ps[:, 0:512], lhsT=xt[:, kk, :],
                        rhs=wq_sb[kk][:, 0:512],
                        start=(kk == 0), stop=(kk == 7))
                    nc.tensor.matmul(
                        qkv_ps[:, 512:1024], lhsT=xt[:, kk, :],
                        rhs=wq_sb[kk][:, 512:1024],
                        start=(kk == 0), stop=(kk == 7))
                    nc.tensor.matmul(
                        v_ps, lhsT=xt[:, kk, :],
                        rhs=wq_sb[kk][:, 1024:1536],
                        start=(kk == 0), stop=(kk == 7))

                # q softmax over dh: one exp, one grouped reduce, one
                # broadcast multiply
                expq = work.tile([128, INNER], F32, tag="expq")
                nc.scalar.activation(out=expq, in_=qkv_ps[:, 0:512], func=EXP)
                qs = small.tile([128, 8], F32, tag="qs")
                nc.vector.tensor_reduce(
                    out=qs, in_=expq[:].rearrange("p (h d) -> p h d", h=H),
                    axis=mybir.AxisListType.X, op=mybir.AluOpType.add)
                rq = small.tile([128, 8], F32, tag="rq")
                nc.vector.reciprocal(rq, qs)
                qsm = work.tile([128, INNER], BF, tag="qsm")
                nc.vector.tensor_tensor(
                    out=qsm[:].rearrange("p (h d) -> p h d", h=H),
                    in0=expq[:].rearrange("p (h d) -> p h d", h=H),
                    in1=rq[:].broadcast_to([128, H, DH]),
                    op=mybir.AluOpType.mult)

                expk = work.tile([128, INNER], BF, tag="expk")
                nc.scalar.activation(out=expk, in_=qkv_ps[:, 512:1024], func=EXP)
                vsb = work.tile([128, INNER], BF, tag="v")
                nc.scalar.activation(out=vsb, in_=v_ps, func=COPY)

                # q_sm^T via PE transposes
                for t4 in range(4):
                    trp = tr_psum.tile([128, 128], BF, tag="tr")
                    nc.tensor.transpose(trp, qsm[:, 128 * t4:128 * (t4 + 1)], ident)
                    nc.vector.tensor_copy(
                        out=qsmT[:, t4, m * 128:(m + 1) * 128], in_=trp)

                # head-pair gram blocks + Z col-sums
                cz = c_psum.tile([128, 512], F32, tag="cz")
                for p in range(4):
                    nc.tensor.matmul(
                        cz[:, 128 * p:128 * (p + 1)],
                        lhsT=vsb[:, 128 * p:128 * (p + 1)],
                        rhs=expk[:, 128 * p:128 * (p + 1)],
                        start=True, stop=True)
                zp = z_psum.tile([128, 4], F32, tag="zp")
                for j in range(4):
                    nc.tensor.matmul(
                        zp[:, j:j + 1],
                        lhsT=expk[:, 128 * j:128 * (j + 1)], rhs=ones_bf,
                        start=True, stop=True)
                nc.vector.tensor_add(cz_acc[:, 0:512], cz_acc[:, 0:512], cz)
                nc.vector.tensor_add(cz_acc[:, 512:516], cz_acc[:, 512:516], zp)

        # CC AllReduce fallback (bisect)
        cz_bf = const.tile([128, 516], BF)
        nc.vector.tensor_copy(out=cz_bf, in_=cz_acc)
        gather = const.tile([128, 3, 516], BF)
        part = dram.tile([128, 516], BF, tag="part")
        redd = dram.tile([128, 516], BF, tag="redd")
        nc.gpsimd.dma_start(out=part, in_=cz_bf)
        nc.gpsimd.collective_compute(
            "AllReduce", mybir.AluOpType.add,
            replica_groups=[[0, 1, 2, 3], [4, 5, 6, 7]],
            ins=[part.opt()], outs=[redd.opt()])

        with ExitStack() as p45:
            work2 = p45.enter_context(tc.tile_pool(name="work2", bufs=2))
            small2 = p45.enter_context(tc.tile_pool(name="small2", bufs=2))
            ysb_pool = p45.enter_context(tc.tile_pool(name="ysb", bufs=4))
            m_psum = p45.enter_context(tc.tile_pool(name="m_ps", bufs=2, space="PSUM"))
            y_psum = p45.enter_context(tc.tile_pool(name="y_ps", bufs=4, space="PSUM"))

            red_bf = work2.tile([128, 516], BF, tag="redbf")
            nc.sync.dma_start(out=red_bf, in_=redd)
            rz = small2.tile([128, 4], F32, tag="rz")
            nc.vector.reciprocal(rz, red_bf[:, 512:516])

            # M_h = (1/Z) * ctx_h @ Wout_h; 1/Z folded into the ACT copy
            m_sb = work2.tile([128, 4, D], BF, tag="msb")
            for p in range(4):
                for cb in range(2):
                    mp = m_psum.tile([128, 512], F32, tag="mp")
                    for j in range(2):
                        nc.tensor.matmul(
                            mp[64 * j:64 * (j + 1), :],
                            lhsT=red_bf[64 * j:64 * (j + 1),
                                        128 * p + 64 * j:128 * p + 64 * (j + 1)],
                            rhs=wo_sb[64 * j:64 * (j + 1), p,
                                      cb * 512:(cb + 1) * 512],
                            start=True, stop=True)
                    nc.scalar.activation(
                        out=m_sb[:, p, cb * 512:(cb + 1) * 512], in_=mp,
                        func=COPY, scale=rz[:, p:p + 1])

            # y = sum_t qsmT_t^T @ M_t
            for mi in range(NT):
                for cb in range(2):
                    yp = y_psum.tile([128, 512], F32, tag="yp")
                    for t in range(4):
                        nc.tensor.matmul(
                            yp, lhsT=qsmT[:, t, mi * 128:(mi + 1) * 128],
                            rhs=m_sb[:, t, cb * 512:(cb + 1) * 512],
                            start=(t == 0), stop=(t == 3))
                    ysb = ysb_pool.tile([128, 512], BF, tag="ysb")
                    if cb == 0:
                        nc.vector.tensor_copy(out=ysb, in_=yp)
                    else:
                        nc.scalar.activation(out=ysb, in_=yp, func=COPY)
                    nc.scalar.dma_start(
                        out=y[mi * 128:(mi + 1) * 128,
                              cb * 512:(cb + 1) * 512],
                        in_=ysb)


def _install_ntff_shim():
    """Provide antenv.axon_hooks when the image lacks it, so
    run_bass_kernel_spmd(trace=True) can capture NTFF profiles."""
    import sys as _sys
    try:
        import antenv.axon_hooks  # noqa: F401
        return
    except ImportError:
        pass
    try:
        import types
        from trn_agent_boot.trn_boot import _ntff_profile_via_ctypes
        hook = _ntff_profile_via_ctypes("/opt/axon/libaxon_pjrt.so")
        mod = types.ModuleType("antenv.axon_hooks")
        mod._hook = hook
        mod.get_axon_ntff_profile_hook = lambda: mod._hook
        def _set(h):
            mod._hook = h
        mod.set_axon_ntff_profile_hook = _set
        _sys.modules["antenv.axon_hooks"] = mod
    except Exception:
        pass


_COMPILED = None


def _build():
    global _COMPILED
    if _COMPILED is None:
        nc = bacc.Bacc("TRN2", target_bir_lowering=False, debug=False,
                       num_devices=NCORES)
        xT = nc.declare_dram_parameter("xT", [D, SEQ], BF, isOutput=False)
        wq = nc.declare_dram_parameter("wq", [D, 3 * INNER], BF, isOutput=False)
        wo = nc.declare_dram_parameter("wo", [INNER, D], BF, isOutput=False)
        y = nc.declare_dram_parameter("y", [SEQ, D], BF, isOutput=True)
        with tile.TileContext(nc) as tc:
            _body(tc, xT, wq, wo, y)
        _neuter_fakes()
        nc.compile()
        _COMPILED = nc
    return _COMPILED


def _make_in_maps(x, W_qkv, W_out):
    wq_bf = np.ascontiguousarray(W_qkv).astype(bf16)
    wo_bf = np.ascontiguousarray(W_out).astype(bf16)
    in_maps = []
    for c in range(NCORES):
        b = c // GROUP
        r0 = (c % GROUP) * SEQ
        xT_bf = np.ascontiguousarray(x[b, r0:r0 + SEQ].T).astype(bf16)
        in_maps.append({"xT": xT_bf, "wq": wq_bf, "wo": wo_bf})
    return in_maps


def _run(x, W_qkv, W_out, b_out, trace=False, **spmd_kwargs):
    nc = _build()
    in_maps = _make_in_maps(x, W_qkv, W_out)
    res = run_bass_kernel_spmd(nc, in_maps, list(range(NCORES)),
                               trace=trace, **spmd_kwargs)
    out = np.empty((B, N, D), np.float32)
    for c in range(NCORES):
        b = c // GROUP
        r0 = (c % GROUP) * SEQ
        out[b, r0:r0 + SEQ] = res.results[c]["y"].astype(np.float32)
    out += np.asarray(b_out, np.float32)[None, None, :]
    return out, res


def kernel(x, W_qkv, W_out, b_out):
    x = np.asarray(x, np.float32)
    out, _ = _run(x, np.asarray(W_qkv, np.float32),
                  np.asarray(W_out, np.float32),
                  np.asarray(b_out, np.float32))
    return out
